# revision 82
# baseline (speedup 1.0000x reference)
"""GAT layer (PyG GATConv-style) on 8 Trainium2 NeuronCores — two-phase design.

Strategy (sharding per hint: nodes by id, edges by destination):
- Phase A (device): each core computes, for its node shard,
  [h | a_src | a_dst] = x_shard @ [W.T | W.T@att_src | W.T@att_dst]  (fp16).
- Host exchange: the per-node h|a_src table (the "all-gather of source-node
  features h" from the hint) is assembled on the host and gathered per edge
  slot into a destination-partition-aligned stream: each destination node owns
  one SBUF partition of its block; its incoming edges lie along the free dim
  with block-group-uniform length Lg. Layout per block is [p, feature, l]
  (edge slot innermost) so every element-wise op keeps a packed innermost
  stride for the DVE 2x/4x fast modes. Shipping h (144B/edge) instead of x
  (512B/edge) cuts DMA ~3.5x and the per-edge matmul work ~17x vs expanding x.
- Phase B (device): logits = a_src + a_dst (broadcast add on GPSIMD),
  exp(leaky_relu(z)) via ACT Prelu+Exp, per-head weighting on DVE, and a
  PSUM-accumulated identity matmul per edge column aggregates
  [h*expe | expe] -> [numerator | denominator]. Finalize (division +
  log_softmax, with the stability shift folded into ACT bias/scale; the model
  bias is folded into h in phase A) runs in block chunks interleaved with the
  main loop to avoid a serial drain tail.

kernel(**inputs) takes FULL inputs and returns the FULL [N, 64] fp32 output.
"""

import math

import numpy as np

import concourse.bacc as bacc
import concourse.tile as tile
from concourse import mybir
from concourse.bass_utils import run_bass_kernel_spmd
from concourse.masks import make_identity

# Problem shape (hardcoded per contract)
N, F, E = 100000, 256, 1600000
H, C = 8, 8
HC = H * C            # 64
TD = HC + H           # 72: [h(64) | a_src(8)]
ED = TD + H           # 80: [h | a_src | a_dst] (phase A output row)
NEG_SLOPE = 0.2

P = 128
NCORES = 8
NB = 98               # blocks per core
NPC = NB * P          # 12544 node slots per core
NSLOT = NCORES * NPC  # 100352 >= N
PAD_ASRC = -200.0     # pad edge slots: a_src = -200 -> expe underflows to 0
EXP_SHIFT = 3.0       # final softmax: exp(z - 3), ln(e^3 * s) == logsumexp

GROUP_ELEMS = 8000   # per-partition fp16 elems per he group (16KB)
MAXB = 7              # blocks per group (psum bank: 7*72 = 504 fp32 <= 512)

f16 = np.float16


# ---------------------------------------------------------------- host prep
def _graph_layout(edge_index):
    """Node->row assignment, per-block edge capacities, group schedule and
    per-edge slot positions. Deterministic given edge_index."""
    src = np.asarray(edge_index[0], dtype=np.int64)
    dst = np.asarray(edge_index[1], dtype=np.int64)
    loop = np.arange(N, dtype=np.int64)
    src = np.concatenate([src, loop])
    dst = np.concatenate([dst, loop])

    deg = np.bincount(dst, minlength=N).astype(np.int64)

    # nodes sorted by degree desc -> global 128-slot blocks dealt round-robin
    # to cores so every core's j-th block has (nearly) equal max degree.
    order = np.argsort(-deg, kind="stable")
    ks = np.arange(NSLOT)
    g = ks // P
    p = ks % P
    c = g % NCORES
    j = g // NCORES
    rows = c * NPC + j * P + p
    row2node = np.full(NSLOT, -1, dtype=np.int64)
    row2node[rows[:N]] = order
    node2row = np.empty(N, dtype=np.int64)
    node2row[order] = rows[:N]

    deg_slot = np.zeros(NSLOT, dtype=np.int64)
    deg_slot[:N] = deg[order]
    degb = deg_slot.reshape(NSLOT // P, P).max(axis=1)
    L_sched = degb.reshape(NB, NCORES).max(axis=1)
    L_sched = np.maximum(L_sched, 1)

    # group consecutive blocks with uniform L (max over group), bounded by
    # MAXB blocks (one psum bank) and GROUP_ELEMS per-partition fp16 elems.
    # The first groups are kept small so compute starts early (pipeline fill)
    # and the last ones small to shorten the drain tail.
    groups = []  # (j0, ng, Lg, elem_off)
    j0 = 0
    off = 0
    while j0 < NB:
        gi = len(groups)
        budget = GROUP_ELEMS // 3 if gi < 1 else GROUP_ELEMS
        ng = 1
        Lg = int(L_sched[j0])
        while j0 + ng < NB and ng < MAXB:
            nl = max(Lg, int(L_sched[j0 + ng]))
            if (ng + 1) * TD * nl > budget:
                break
            Lg = nl
            ng += 1
        groups.append((j0, ng, Lg, off))
        off += ng * TD * Lg
        j0 += ng
    TOT = off

    L_uni = np.empty(NB, dtype=np.int64)      # uniform L of each block
    BOFF = np.empty(NB, dtype=np.int64)       # he elem offset of each block
    LOFF = np.empty(NB + 1, dtype=np.int64)   # slot-column offset per block
    lo = 0
    for (j0, ng, Lg, off) in groups:
        for k in range(ng):
            L_uni[j0 + k] = Lg
            BOFF[j0 + k] = off + k * TD * Lg
            LOFF[j0 + k] = lo
            lo += Lg
    LOFF[NB] = lo
    TOTL = lo
    L_real = L_sched  # true max occupied slots per block (<= L_uni)

    # per-edge slot position (edges sorted by destination)
    eorder = np.argsort(dst, kind="stable")
    dst_s = dst[eorder]
    src_s = src[eorder]
    starts = np.zeros(N + 1, dtype=np.int64)
    starts[1:] = np.cumsum(deg)
    l_rank = np.arange(len(dst_s), dtype=np.int64) - starts[dst_s]
    r = node2row[dst_s]
    ec = r // NPC
    ej = (r % NPC) // P
    ep = r % P

    # per-core index matrix IDX[p, slotcol] = h-table row of the edge source
    idxs = []
    for cc in range(NCORES):
        m = ec == cc
        IDX = np.full((P, TOTL), NSLOT, dtype=np.int64)  # NSLOT = pad row
        flat = ep[m] * TOTL + LOFF[ej[m]] + l_rank[m]
        IDX.reshape(-1)[flat] = node2row[src_s[m]]
        idxs.append(IDX)

    return dict(row2node=row2node, node2row=node2row, groups=groups, TOT=TOT,
                L_uni=L_uni, BOFF=BOFF, LOFF=LOFF, idxs=idxs, L_real=L_real)


def _fold_weights(W, att_src, att_dst, bias):
    Wt = np.asarray(W, dtype=np.float64).T                 # [256, 64]
    att_s = np.asarray(att_src, np.float64)
    att_d = np.asarray(att_dst, np.float64)
    Wts = np.stack([Wt[:, h * C:(h + 1) * C] @ att_s[h] for h in range(H)], axis=1)
    Wtd = np.stack([Wt[:, h * C:(h + 1) * C] @ att_d[h] for h in range(H)], axis=1)
    Wext = np.concatenate([Wt, Wts, Wtd], axis=1)          # [256, 80]
    # Row 256 (matched with an all-ones x row) adds bias to every h row:
    # since the attention weights of each destination sum to 1, aggregating
    # h+bias yields exactly out+bias — the bias add is folded into phase A.
    brow = np.zeros((1, ED), dtype=np.float64)
    brow[0, :HC] = np.asarray(bias, np.float64)
    Wext = np.concatenate([Wext, brow], axis=0)            # [257, 80]
    return np.ascontiguousarray(Wext.astype(np.float32).astype(f16))


# ---------------------------------------------------------------- programs
def _build_phase_a():
    nc = bacc.Bacc("TRN2", target_bir_lowering=False, debug=False,
                   enable_asserts=False, num_devices=NCORES)
    dt = mybir.dt
    AF = mybir.ActivationFunctionType

    xoT = nc.dram_tensor("xoT", [F + 1, NPC], dt.float16, kind="ExternalInput").ap()
    Wext = nc.dram_tensor("Wext", [F + 1, ED], dt.float16, kind="ExternalInput").ap()
    ha = nc.dram_tensor("ha", [P, NB * ED], dt.float16, kind="ExternalOutput").ap()

    XCB = 14      # blocks of x per DMA chunk
    PSB = 6       # blocks per psum bank (6*80 = 480 fp32 <= 512)

    with tile.TileContext(nc) as tc:
        with (
            tc.tile_pool(name="const", bufs=1) as constp,
            tc.tile_pool(name="resid", bufs=1) as residp,
            tc.tile_pool(name="xp", bufs=3) as xp,
            tc.tile_pool(name="ps", bufs=3, space="PSUM") as psp,
        ):
            # first x chunk goes out before the constants so the DMA engines
            # start on the long pole immediately
            w0 = XCB * P
            xt = xp.tile([P, 2, w0], dt.float16, tag="xt")
            nc.sync.dma_start(xt[:], xoT[0:2 * P, 0:w0].rearrange(
                "(a p) m -> p a m", a=2))
            wt01 = constp.tile([P, 2, ED], dt.float16)
            nc.sync.dma_start(wt01[:], Wext[0:2 * P, :].rearrange(
                "(a p) d -> p a d", a=2))
            wt0 = wt01[:, 0, :]
            wt1 = wt01[:, 1, :]
            brow = constp.tile([1, ED], dt.float16)
            nc.sync.dma_start(brow[:], Wext[2 * P:2 * P + 1, :])
            ones_t = constp.tile([1, NPC], dt.float16)
            nc.sync.dma_start(ones_t[:], xoT[F:F + 1, :])
            ha_sb = residp.tile([P, NB * ED], dt.float16)

            aps = None
            xbase = 0
            for jb in range(NB):
                if jb in (14, 28, 42, 56, 70, 84, 91):
                    nxt = {14: 28, 28: 42, 42: 56, 56: 70, 70: 84,
                           84: 91, 91: 98}[jb]
                    w = (nxt - jb) * P
                    xt = xp.tile([P, 2, w], dt.float16, tag="xt")
                    nc.sync.dma_start(xt[:], xoT[0:2 * P, jb * P:jb * P + w]
                                      .rearrange("(a p) m -> p a m", a=2))
                    xbase = jb
                if jb % PSB == 0:
                    nb_ps = min(PSB, NB - jb)
                    aps = psp.tile([P, nb_ps * ED], dt.float32, space="PSUM", tag="aps")
                ko = (jb - (xbase if jb >= 14 else 0)) * P
                po = (jb % PSB) * ED
                nc.tensor.matmul(aps[:, po:po + ED], lhsT=xt[:, 0, ko:ko + P],
                                 rhs=wt0, start=True, stop=False,
                                 skip_group_check=True)
                nc.tensor.matmul(aps[:, po:po + ED], lhsT=xt[:, 1, ko:ko + P],
                                 rhs=wt1, start=False, stop=False,
                                 skip_group_check=True)
                nc.tensor.matmul(aps[:, po:po + ED],
                                 lhsT=ones_t[:, jb * P:(jb + 1) * P],
                                 rhs=brow[:], start=False, stop=True,
                                 skip_group_check=True)
                if jb % PSB == PSB - 1 or jb == NB - 1:
                    g0 = (jb // PSB) * PSB
                    nw = (jb - g0 + 1) * ED
                    with nc.allow_low_precision(reason="f16 h table"):
                        nc.scalar.activation(ha_sb[:, g0 * ED:g0 * ED + nw],
                                             aps[:, 0:nw], AF.Copy)
                if jb in (23, 47, 71, 89, 97):
                    bset = (23, 47, 71, 89, 97)
                    prev = ([-1] + list(bset))[bset.index(jb)]
                    o0 = (prev + 1) * ED
                    nc.sync.dma_start(ha[:, o0:(jb + 1) * ED],
                                      ha_sb[:, o0:(jb + 1) * ED])
    nc.compile()
    return nc


def _build_phase_b(groups, TOT, L_real):
    nc = bacc.Bacc("TRN2", target_bir_lowering=False, debug=False,
                   enable_asserts=False, num_devices=NCORES)
    dt = mybir.dt
    AF = mybir.ActivationFunctionType
    OP = mybir.AluOpType

    he = nc.dram_tensor("he", [P, TOT], dt.float16, kind="ExternalInput").ap()
    adst = nc.dram_tensor("adst", [P, NB * H], dt.float16, kind="ExternalInput").ap()
    outp = nc.dram_tensor("outp", [P, NB * HC], dt.float16, kind="ExternalOutput").ap()

    with tile.TileContext(nc) as tc:
        with (
            tc.tile_pool(name="const", bufs=1) as constp,
            tc.tile_pool(name="resid", bufs=1) as residp,
            tc.tile_pool(name="hep", bufs=4) as hep,
            tc.tile_pool(name="wp", bufs=4) as wp,
            tc.tile_pool(name="lop", bufs=3) as lop,
            tc.tile_pool(name="psp", bufs=4, space="PSUM") as psp,
            tc.tile_pool(name="fin", bufs=3) as finp,
        ):
            ident = constp.tile([P, P], dt.float16)
            make_identity(nc, ident[:])
            adst_t = constp.tile([P, NB * H], dt.float16)
            nc.sync.dma_start(adst_t[:], adst[:, :])
            shift_t = constp.tile([P, 1], dt.float32)
            nc.gpsimd.memset(shift_t[:], -EXP_SHIFT)
            nums = residp.tile([P, NB * TD], dt.float16)
            numsv = nums[:].rearrange("p (j d) -> p j d", d=TD)

            def emit_finalize(b0, b1):
                """log_softmax(num/s) for block range [b0, b1); bias is
                already folded into h (phase A). All tiles are chunk-local
                scratch from the finp pool."""
                nb = b1 - b0
                srec = finp.tile([P, nb, H], dt.float16, tag="srec")
                ob = finp.tile([P, nb, HC], dt.float16, tag="ob")
                exf = finp.tile([P, nb, HC], dt.float16, tag="exf")
                sm = finp.tile([P, nb], dt.float16, tag="sm")
                lnt = finp.tile([P, nb], dt.float16, tag="lnt")
                with nc.allow_low_precision(reason="f16 1/s"):
                    nc.vector.reciprocal(srec[:],
                                         numsv[:, b0:b1, HC:TD])
                nc.vector.tensor_tensor(
                    out=ob[:].rearrange("p j (h c) -> p j h c", c=C),
                    in0=numsv[:, b0:b1, 0:HC].rearrange("p j (h c) -> p j h c", c=C),
                    in1=srec[:].unsqueeze(3).to_broadcast([P, nb, H, C]),
                    op=OP.mult)
                nc.scalar.activation(exf[:], ob[:], AF.Exp, bias=shift_t[:])
                with nc.allow_low_precision(reason="f16 softmax sum"):
                    nc.vector.tensor_tensor(
                        out=exf[:, :, 0:HC // 2], in0=exf[:, :, 0:HC // 2],
                        in1=exf[:, :, HC // 2:HC], op=OP.add)
                    nc.vector.tensor_reduce(sm[:], exf[:, :, 0:HC // 2],
                                            axis=mybir.AxisListType.X, op=OP.add)
                nc.scalar.activation(lnt[:], sm[:], AF.Ln,
                                     scale=math.exp(EXP_SHIFT))
                with nc.allow_low_precision(reason="f16 output"):
                    nc.vector.tensor_tensor(
                        out=ob[:], in0=ob[:],
                        in1=lnt[:].unsqueeze(2).to_broadcast([P, nb, HC]),
                        op=OP.subtract)
                nc.sync.dma_start(outp[:, b0 * HC:b1 * HC],
                                  ob[:].rearrange("p j d -> p (j d)"))

            NGR = len(groups)
            fin_after = {}  # group idx -> (b0, b1): finalize ready block range
            done_after = [groups[g][0] + groups[g][1] for g in range(NGR)]
            # finalize in chunks as soon as blocks are ready; small last chunk
            # keeps the drain tail short
            tgt = [22, 44, 64, 80, 92]
            b0 = 0
            ti = 0
            for g in range(NGR):
                # one group of slack so finalize doesn't head-of-line block
                if g >= 2 and ti < len(tgt) and done_after[g - 2] >= tgt[ti]:
                    fin_after[g] = (b0, done_after[g - 2])
                    b0 = done_after[g - 2]
                    ti += 1

            for g, (j0, ng, Lg, off) in enumerate(groups):
                CW = ng * TD * Lg
                heg = hep.tile([P, CW], dt.float16, tag="he")
                nc.sync.dma_start(heg[:], he[:, off:off + CW])
                hev = heg[:].rearrange("p (a d l) -> p a d l", d=TD, l=Lg)

                lo = lop.tile([P, ng, H, Lg], dt.float16, tag="lo")
                lo_eng = nc.vector if g < 3 else nc.gpsimd
                lo_eng.tensor_tensor(
                    out=lo[:], in0=hev[:, :, HC:TD, :],
                    in1=adst_t[:].rearrange("p (j h) -> p j h", h=H)
                        [:, j0:j0 + ng, :].unsqueeze(3).to_broadcast([P, ng, H, Lg]),
                    op=OP.add)
                lk = lop.tile([P, ng, H, Lg], dt.float16, tag="lk")
                nc.scalar.activation(lk[:], lo[:], AF.Prelu, alpha=NEG_SLOPE)
                wg = wp.tile([P, CW], dt.float16, tag="w")
                wv = wg[:].rearrange("p (a d l) -> p a d l", d=TD, l=Lg)
                nc.scalar.activation(wv[:, :, HC:TD, :], lk[:], AF.Exp)
                for h in range(H):
                    nc.vector.tensor_tensor(
                        out=wv[:, :, h * C:(h + 1) * C, :],
                        in0=hev[:, :, h * C:(h + 1) * C, :],
                        in1=wv[:, :, HC + h, :].unsqueeze(2)
                            .to_broadcast([P, ng, C, Lg]),
                        op=OP.mult)

                aps = psp.tile([P, ng * TD], dt.float32, space="PSUM", tag="agg")
                for k in range(ng):
                    Lr = int(L_real[j0 + k])
                    for l in range(Lr):
                        nc.tensor.matmul(aps[:, k * TD:(k + 1) * TD],
                                         lhsT=ident[:], rhs=wv[:, k, :, l],
                                         start=(l == 0), stop=(l == Lr - 1),
                                         skip_group_check=True)
                with nc.allow_low_precision(reason="f16 block aggregates"):
                    nc.scalar.activation(nums[:, j0 * TD:(j0 + ng) * TD],
                                         aps[:], AF.Copy)
                if g in fin_after:
                    emit_finalize(*fin_after[g])

            emit_finalize(b0, NB)
    nc.compile()
    return nc


# ---------------------------------------------------------------- kernel
def _prep_phase_a_inputs(x, lay, Wext):
    x16 = np.asarray(x, np.float32).astype(f16)
    in_maps = []
    row2node = lay["row2node"]
    for cc in range(NCORES):
        rr = row2node[cc * NPC:(cc + 1) * NPC]
        m = rr >= 0
        xoT = np.empty((F + 1, NPC), dtype=f16)
        xo = np.zeros((NPC, F), dtype=f16)
        xo[m] = x16[rr[m]]
        xoT[:F] = xo.T
        xoT[F] = 1.0           # ones row pairs with the bias row of Wext
        in_maps.append({"xoT": xoT, "Wext": Wext})
    return in_maps


def _prep_phase_b_inputs(ha_res, lay):
    TOT = lay["TOT"]
    L_uni, BOFF, LOFF = lay["L_uni"], lay["BOFF"], lay["LOFF"]

    h_all = np.empty((NSLOT + 1, TD), dtype=f16)
    adsts = []
    for cc in range(NCORES):
        ha3 = ha_res[cc]["ha"].reshape(P, NB, ED)
        h_all[cc * NPC:(cc + 1) * NPC] = (
            ha3[:, :, :TD].transpose(1, 0, 2).reshape(NPC, TD))
        adsts.append(np.ascontiguousarray(
            ha3[:, :, TD:ED].reshape(P, NB * H)))
    h_all[NSLOT, :HC] = 0.0
    h_all[NSLOT, HC:] = PAD_ASRC

    in_maps = []
    for cc in range(NCORES):
        IDX = lay["idxs"][cc]
        he = np.empty((P, TOT), dtype=f16)
        for jb in range(NB):
            Lg = int(L_uni[jb])
            blk = h_all[IDX[:, LOFF[jb]:LOFF[jb + 1]]]       # [P, Lg, TD]
            he[:, BOFF[jb]:BOFF[jb] + TD * Lg] = (
                blk.transpose(0, 2, 1).reshape(P, TD * Lg))
        in_maps.append({"he": he, "adst": adsts[cc]})
    return in_maps


def _unshard_output(res, lay):
    out_full = np.empty((N, HC), dtype=np.float32)
    row2node = lay["row2node"]
    for cc in range(NCORES):
        o3 = (res[cc]["outp"].astype(np.float32)
              .reshape(P, NB, HC).transpose(1, 0, 2).reshape(NPC, HC))
        rr = row2node[cc * NPC:(cc + 1) * NPC]
        m = rr >= 0
        out_full[rr[m]] = o3[m]
    return out_full


def kernel_with_programs(x, edge_index, W, att_src, att_dst, bias):
    lay = _graph_layout(edge_index)
    Wext = _fold_weights(W, att_src, att_dst, bias)

    nc_a = _build_phase_a()
    maps_a = _prep_phase_a_inputs(x, lay, Wext)
    res_a = run_bass_kernel_spmd(nc_a, maps_a, core_ids=list(range(NCORES)))

    nc_b = _build_phase_b(lay["groups"], lay["TOT"], lay["L_real"])
    maps_b = _prep_phase_b_inputs(res_a.results, lay)
    res_b = run_bass_kernel_spmd(nc_b, maps_b, core_ids=list(range(NCORES)))

    return _unshard_output(res_b.results, lay), (nc_a, nc_b)


def kernel(x, edge_index, W, att_src, att_dst, bias):
    out, _ = kernel_with_programs(x, edge_index, W, att_src, att_dst, bias)
    return out


# revision 99
# speedup vs baseline: 1.0010x; 1.0010x over previous
"""GAT layer (PyG GATConv-style) on 8 Trainium2 NeuronCores — two-phase design.

Strategy (sharding per hint: nodes by id, edges by destination):
- Phase A (device): each core computes, for its node shard,
  [h | a_src | a_dst] = x_shard @ [W.T | W.T@att_src | W.T@att_dst]  (fp16).
- Host exchange: the per-node h|a_src table (the "all-gather of source-node
  features h" from the hint) is assembled on the host and gathered per edge
  slot into a destination-partition-aligned stream: each destination node owns
  one SBUF partition of its block; its incoming edges lie along the free dim
  with block-group-uniform length Lg. Layout per block is [p, feature, l]
  (edge slot innermost) so every element-wise op keeps a packed innermost
  stride for the DVE 2x/4x fast modes. Shipping h (144B/edge) instead of x
  (512B/edge) cuts DMA ~3.5x and the per-edge matmul work ~17x vs expanding x.
- Phase B (device): logits = a_src + a_dst (broadcast add on GPSIMD),
  exp(leaky_relu(z)) via ACT Prelu+Exp, per-head weighting on DVE, and a
  PSUM-accumulated identity matmul per edge column aggregates
  [h*expe | expe] -> [numerator | denominator]. Finalize (division +
  log_softmax, with the stability shift folded into ACT bias/scale; the model
  bias is folded into h in phase A) runs in block chunks interleaved with the
  main loop to avoid a serial drain tail.

kernel(**inputs) takes FULL inputs and returns the FULL [N, 64] fp32 output.
"""

import math

import numpy as np

import concourse.bacc as bacc
import concourse.tile as tile
from concourse import mybir
from concourse.bass_utils import run_bass_kernel_spmd
from concourse.masks import make_identity

# Problem shape (hardcoded per contract)
N, F, E = 100000, 256, 1600000
H, C = 8, 8
HC = H * C            # 64
TD = HC + H           # 72: [h(64) | a_src(8)]
ED = TD + H           # 80: [h | a_src | a_dst] (phase A output row)
NEG_SLOPE = 0.2

P = 128
NCORES = 8
NB = 98               # blocks per core
NPC = NB * P          # 12544 node slots per core
NSLOT = NCORES * NPC  # 100352 >= N
PAD_ASRC = -200.0     # pad edge slots: a_src = -200 -> expe underflows to 0
EXP_SHIFT = 3.0       # final softmax: exp(z - 3), ln(e^3 * s) == logsumexp

GROUP_ELEMS = 8500   # per-partition fp16 elems per he group (16KB)
MAXB = 7              # blocks per group (psum bank: 7*72 = 504 fp32 <= 512)

f16 = np.float16


# ---------------------------------------------------------------- host prep
def _graph_layout(edge_index):
    """Node->row assignment, per-block edge capacities, group schedule and
    per-edge slot positions. Deterministic given edge_index."""
    src = np.asarray(edge_index[0], dtype=np.int64)
    dst = np.asarray(edge_index[1], dtype=np.int64)
    loop = np.arange(N, dtype=np.int64)
    src = np.concatenate([src, loop])
    dst = np.concatenate([dst, loop])

    deg = np.bincount(dst, minlength=N).astype(np.int64)

    # nodes sorted by degree desc -> global 128-slot blocks dealt round-robin
    # to cores so every core's j-th block has (nearly) equal max degree.
    order = np.argsort(-deg, kind="stable")
    ks = np.arange(NSLOT)
    g = ks // P
    p = ks % P
    c = g % NCORES
    j = g // NCORES
    rows = c * NPC + j * P + p
    row2node = np.full(NSLOT, -1, dtype=np.int64)
    row2node[rows[:N]] = order
    node2row = np.empty(N, dtype=np.int64)
    node2row[order] = rows[:N]

    deg_slot = np.zeros(NSLOT, dtype=np.int64)
    deg_slot[:N] = deg[order]
    degb = deg_slot.reshape(NSLOT // P, P).max(axis=1)
    L_sched = degb.reshape(NB, NCORES).max(axis=1)
    L_sched = np.maximum(L_sched, 1)

    # group consecutive blocks with uniform L (max over group), bounded by
    # MAXB blocks (one psum bank) and GROUP_ELEMS per-partition fp16 elems.
    # The first groups are kept small so compute starts early (pipeline fill)
    # and the last ones small to shorten the drain tail.
    groups = []  # (j0, ng, Lg, elem_off)
    j0 = 0
    off = 0
    while j0 < NB:
        gi = len(groups)
        budget = GROUP_ELEMS // 3 if gi < 1 else GROUP_ELEMS
        ng = 1
        Lg = int(L_sched[j0])
        while j0 + ng < NB and ng < MAXB:
            nl = max(Lg, int(L_sched[j0 + ng]))
            if (ng + 1) * TD * nl > budget:
                break
            Lg = nl
            ng += 1
        groups.append((j0, ng, Lg, off))
        off += ng * TD * Lg
        j0 += ng
    TOT = off

    L_uni = np.empty(NB, dtype=np.int64)      # uniform L of each block
    BOFF = np.empty(NB, dtype=np.int64)       # he elem offset of each block
    LOFF = np.empty(NB + 1, dtype=np.int64)   # slot-column offset per block
    lo = 0
    for (j0, ng, Lg, off) in groups:
        for k in range(ng):
            L_uni[j0 + k] = Lg
            BOFF[j0 + k] = off + k * TD * Lg
            LOFF[j0 + k] = lo
            lo += Lg
    LOFF[NB] = lo
    TOTL = lo
    L_real = L_sched  # true max occupied slots per block (<= L_uni)

    # per-edge slot position (edges sorted by destination)
    eorder = np.argsort(dst, kind="stable")
    dst_s = dst[eorder]
    src_s = src[eorder]
    starts = np.zeros(N + 1, dtype=np.int64)
    starts[1:] = np.cumsum(deg)
    l_rank = np.arange(len(dst_s), dtype=np.int64) - starts[dst_s]
    r = node2row[dst_s]
    ec = r // NPC
    ej = (r % NPC) // P
    ep = r % P

    # per-core index matrix IDX[p, slotcol] = h-table row of the edge source
    idxs = []
    for cc in range(NCORES):
        m = ec == cc
        IDX = np.full((P, TOTL), NSLOT, dtype=np.int64)  # NSLOT = pad row
        flat = ep[m] * TOTL + LOFF[ej[m]] + l_rank[m]
        IDX.reshape(-1)[flat] = node2row[src_s[m]]
        idxs.append(IDX)

    return dict(row2node=row2node, node2row=node2row, groups=groups, TOT=TOT,
                L_uni=L_uni, BOFF=BOFF, LOFF=LOFF, idxs=idxs, L_real=L_real)


def _fold_weights(W, att_src, att_dst, bias):
    Wt = np.asarray(W, dtype=np.float64).T                 # [256, 64]
    att_s = np.asarray(att_src, np.float64)
    att_d = np.asarray(att_dst, np.float64)
    Wts = np.stack([Wt[:, h * C:(h + 1) * C] @ att_s[h] for h in range(H)], axis=1)
    Wtd = np.stack([Wt[:, h * C:(h + 1) * C] @ att_d[h] for h in range(H)], axis=1)
    Wext = np.concatenate([Wt, Wts, Wtd], axis=1)          # [256, 80]
    # Row 256 (matched with an all-ones x row) adds bias to every h row:
    # since the attention weights of each destination sum to 1, aggregating
    # h+bias yields exactly out+bias — the bias add is folded into phase A.
    brow = np.zeros((1, ED), dtype=np.float64)
    brow[0, :HC] = np.asarray(bias, np.float64)
    Wext = np.concatenate([Wext, brow], axis=0)            # [257, 80]
    return np.ascontiguousarray(Wext.astype(np.float32).astype(f16))


# ---------------------------------------------------------------- programs
def _build_phase_a():
    nc = bacc.Bacc("TRN2", target_bir_lowering=False, debug=False,
                   enable_asserts=False, num_devices=NCORES)
    dt = mybir.dt
    AF = mybir.ActivationFunctionType

    xoT = nc.dram_tensor("xoT", [F + 1, NPC], dt.float16, kind="ExternalInput").ap()
    Wext = nc.dram_tensor("Wext", [F + 1, ED], dt.float16, kind="ExternalInput").ap()
    ha = nc.dram_tensor("ha", [P, NB * ED], dt.float16, kind="ExternalOutput").ap()

    XCB = 14      # blocks of x per DMA chunk
    PSB = 6       # blocks per psum bank (6*80 = 480 fp32 <= 512)

    with tile.TileContext(nc) as tc:
        with (
            tc.tile_pool(name="const", bufs=1) as constp,
            tc.tile_pool(name="resid", bufs=1) as residp,
            tc.tile_pool(name="xp", bufs=3) as xp,
            tc.tile_pool(name="ps", bufs=3, space="PSUM") as psp,
        ):
            # first x chunk goes out before the constants so the DMA engines
            # start on the long pole immediately
            w0 = XCB * P
            xt = xp.tile([P, 2, w0], dt.float16, tag="xt")
            nc.sync.dma_start(xt[:], xoT[0:2 * P, 0:w0].rearrange(
                "(a p) m -> p a m", a=2))
            wt01 = constp.tile([P, 2, ED], dt.float16)
            nc.sync.dma_start(wt01[:], Wext[0:2 * P, :].rearrange(
                "(a p) d -> p a d", a=2))
            wt0 = wt01[:, 0, :]
            wt1 = wt01[:, 1, :]
            brow = constp.tile([1, ED], dt.float16)
            nc.sync.dma_start(brow[:], Wext[2 * P:2 * P + 1, :])
            ones_t = constp.tile([1, NPC], dt.float16)
            nc.sync.dma_start(ones_t[:], xoT[F:F + 1, :])
            ha_sb = residp.tile([P, NB * ED], dt.float16)

            aps = None
            xbase = 0
            for jb in range(NB):
                if jb in (14, 28, 42, 56, 70, 84, 91):
                    nxt = {14: 28, 28: 42, 42: 56, 56: 70, 70: 84,
                           84: 91, 91: 98}[jb]
                    w = (nxt - jb) * P
                    xt = xp.tile([P, 2, w], dt.float16, tag="xt")
                    nc.sync.dma_start(xt[:], xoT[0:2 * P, jb * P:jb * P + w]
                                      .rearrange("(a p) m -> p a m", a=2))
                    xbase = jb
                if jb % PSB == 0:
                    nb_ps = min(PSB, NB - jb)
                    aps = psp.tile([P, nb_ps * ED], dt.float32, space="PSUM", tag="aps")
                ko = (jb - (xbase if jb >= 14 else 0)) * P
                po = (jb % PSB) * ED
                nc.tensor.matmul(aps[:, po:po + ED], lhsT=xt[:, 0, ko:ko + P],
                                 rhs=wt0, start=True, stop=False,
                                 skip_group_check=True)
                nc.tensor.matmul(aps[:, po:po + ED], lhsT=xt[:, 1, ko:ko + P],
                                 rhs=wt1, start=False, stop=False,
                                 skip_group_check=True)
                nc.tensor.matmul(aps[:, po:po + ED],
                                 lhsT=ones_t[:, jb * P:(jb + 1) * P],
                                 rhs=brow[:], start=False, stop=True,
                                 skip_group_check=True)
                if jb % PSB == PSB - 1 or jb == NB - 1:
                    g0 = (jb // PSB) * PSB
                    nw = (jb - g0 + 1) * ED
                    with nc.allow_low_precision(reason="f16 h table"):
                        nc.scalar.activation(ha_sb[:, g0 * ED:g0 * ED + nw],
                                             aps[:, 0:nw], AF.Copy)
                if jb in (23, 47, 71, 89, 97):
                    bset = (23, 47, 71, 89, 97)
                    prev = ([-1] + list(bset))[bset.index(jb)]
                    o0 = (prev + 1) * ED
                    nc.sync.dma_start(ha[:, o0:(jb + 1) * ED],
                                      ha_sb[:, o0:(jb + 1) * ED])
    nc.compile()
    return nc


def _build_phase_b(groups, TOT, L_real):
    nc = bacc.Bacc("TRN2", target_bir_lowering=False, debug=False,
                   enable_asserts=False, num_devices=NCORES)
    dt = mybir.dt
    AF = mybir.ActivationFunctionType
    OP = mybir.AluOpType

    he = nc.dram_tensor("he", [P, TOT], dt.float16, kind="ExternalInput").ap()
    adst = nc.dram_tensor("adst", [P, NB * H], dt.float16, kind="ExternalInput").ap()
    outp = nc.dram_tensor("outp", [P, NB * HC], dt.float16, kind="ExternalOutput").ap()

    with tile.TileContext(nc) as tc:
        with (
            tc.tile_pool(name="const", bufs=1) as constp,
            tc.tile_pool(name="resid", bufs=1) as residp,
            tc.tile_pool(name="hep", bufs=4) as hep,
            tc.tile_pool(name="wp", bufs=4) as wp,
            tc.tile_pool(name="lop", bufs=3) as lop,
            tc.tile_pool(name="psp", bufs=4, space="PSUM") as psp,
            tc.tile_pool(name="fin", bufs=3) as finp,
        ):
            ident = constp.tile([P, P], dt.float16)
            make_identity(nc, ident[:])
            adst_t = constp.tile([P, NB * H], dt.float16)
            nc.sync.dma_start(adst_t[:], adst[:, :])
            shift_t = constp.tile([P, 1], dt.float32)
            nc.gpsimd.memset(shift_t[:], -EXP_SHIFT)
            nums = residp.tile([P, NB * TD], dt.float16)
            numsv = nums[:].rearrange("p (j d) -> p j d", d=TD)

            def emit_finalize(b0, b1):
                """log_softmax(num/s) for block range [b0, b1); bias is
                already folded into h (phase A). All tiles are chunk-local
                scratch from the finp pool."""
                nb = b1 - b0
                srec = finp.tile([P, nb, H], dt.float16, tag="srec")
                ob = finp.tile([P, nb, HC], dt.float16, tag="ob")
                exf = finp.tile([P, nb, HC], dt.float16, tag="exf")
                sm = finp.tile([P, nb], dt.float16, tag="sm")
                lnt = finp.tile([P, nb], dt.float16, tag="lnt")
                with nc.allow_low_precision(reason="f16 1/s"):
                    nc.vector.reciprocal(srec[:],
                                         numsv[:, b0:b1, HC:TD])
                nc.vector.tensor_tensor(
                    out=ob[:].rearrange("p j (h c) -> p j h c", c=C),
                    in0=numsv[:, b0:b1, 0:HC].rearrange("p j (h c) -> p j h c", c=C),
                    in1=srec[:].unsqueeze(3).to_broadcast([P, nb, H, C]),
                    op=OP.mult)
                nc.scalar.activation(exf[:], ob[:], AF.Exp, bias=shift_t[:])
                with nc.allow_low_precision(reason="f16 softmax sum"):
                    nc.vector.tensor_tensor(
                        out=exf[:, :, 0:HC // 2], in0=exf[:, :, 0:HC // 2],
                        in1=exf[:, :, HC // 2:HC], op=OP.add)
                    nc.vector.tensor_reduce(sm[:], exf[:, :, 0:HC // 2],
                                            axis=mybir.AxisListType.X, op=OP.add)
                nc.scalar.activation(lnt[:], sm[:], AF.Ln,
                                     scale=math.exp(EXP_SHIFT))
                with nc.allow_low_precision(reason="f16 output"):
                    nc.vector.tensor_tensor(
                        out=ob[:], in0=ob[:],
                        in1=lnt[:].unsqueeze(2).to_broadcast([P, nb, HC]),
                        op=OP.subtract)
                nc.sync.dma_start(outp[:, b0 * HC:b1 * HC],
                                  ob[:].rearrange("p j d -> p (j d)"))

            NGR = len(groups)
            fin_after = {}  # group idx -> (b0, b1): finalize ready block range
            done_after = [groups[g][0] + groups[g][1] for g in range(NGR)]
            # finalize in chunks as soon as blocks are ready; small last chunk
            # keeps the drain tail short
            tgt = [22, 44, 64, 80, 92]
            b0 = 0
            ti = 0
            for g in range(NGR):
                # one group of slack so finalize doesn't head-of-line block
                if g >= 2 and ti < len(tgt) and done_after[g - 2] >= tgt[ti]:
                    fin_after[g] = (b0, done_after[g - 2])
                    b0 = done_after[g - 2]
                    ti += 1

            for g, (j0, ng, Lg, off) in enumerate(groups):
                CW = ng * TD * Lg
                heg = hep.tile([P, CW], dt.float16, tag="he")
                nc.sync.dma_start(heg[:], he[:, off:off + CW])
                hev = heg[:].rearrange("p (a d l) -> p a d l", d=TD, l=Lg)

                lo = lop.tile([P, ng, H, Lg], dt.float16, tag="lo")
                lo_eng = nc.vector if g < 3 else nc.gpsimd
                lo_eng.tensor_tensor(
                    out=lo[:], in0=hev[:, :, HC:TD, :],
                    in1=adst_t[:].rearrange("p (j h) -> p j h", h=H)
                        [:, j0:j0 + ng, :].unsqueeze(3).to_broadcast([P, ng, H, Lg]),
                    op=OP.add)
                lk = lop.tile([P, ng, H, Lg], dt.float16, tag="lk")
                nc.scalar.activation(lk[:], lo[:], AF.Prelu, alpha=NEG_SLOPE)
                wg = wp.tile([P, CW], dt.float16, tag="w")
                wv = wg[:].rearrange("p (a d l) -> p a d l", d=TD, l=Lg)
                nc.scalar.activation(wv[:, :, HC:TD, :], lk[:], AF.Exp)
                for h in range(H):
                    nc.vector.tensor_tensor(
                        out=wv[:, :, h * C:(h + 1) * C, :],
                        in0=hev[:, :, h * C:(h + 1) * C, :],
                        in1=wv[:, :, HC + h, :].unsqueeze(2)
                            .to_broadcast([P, ng, C, Lg]),
                        op=OP.mult)

                aps = psp.tile([P, ng * TD], dt.float32, space="PSUM", tag="agg")
                for k in range(ng):
                    Lr = int(L_real[j0 + k])
                    for l in range(Lr):
                        nc.tensor.matmul(aps[:, k * TD:(k + 1) * TD],
                                         lhsT=ident[:], rhs=wv[:, k, :, l],
                                         start=(l == 0), stop=(l == Lr - 1),
                                         skip_group_check=True)
                with nc.allow_low_precision(reason="f16 block aggregates"):
                    nc.scalar.activation(nums[:, j0 * TD:(j0 + ng) * TD],
                                         aps[:], AF.Copy)
                if g in fin_after:
                    emit_finalize(*fin_after[g])

            emit_finalize(b0, NB)
    nc.compile()
    return nc


# ---------------------------------------------------------------- kernel
def _prep_phase_a_inputs(x, lay, Wext):
    x16 = np.asarray(x, np.float32).astype(f16)
    in_maps = []
    row2node = lay["row2node"]
    for cc in range(NCORES):
        rr = row2node[cc * NPC:(cc + 1) * NPC]
        m = rr >= 0
        xoT = np.empty((F + 1, NPC), dtype=f16)
        xo = np.zeros((NPC, F), dtype=f16)
        xo[m] = x16[rr[m]]
        xoT[:F] = xo.T
        xoT[F] = 1.0           # ones row pairs with the bias row of Wext
        in_maps.append({"xoT": xoT, "Wext": Wext})
    return in_maps


def _prep_phase_b_inputs(ha_res, lay):
    TOT = lay["TOT"]
    L_uni, BOFF, LOFF = lay["L_uni"], lay["BOFF"], lay["LOFF"]

    h_all = np.empty((NSLOT + 1, TD), dtype=f16)
    adsts = []
    for cc in range(NCORES):
        ha3 = ha_res[cc]["ha"].reshape(P, NB, ED)
        h_all[cc * NPC:(cc + 1) * NPC] = (
            ha3[:, :, :TD].transpose(1, 0, 2).reshape(NPC, TD))
        adsts.append(np.ascontiguousarray(
            ha3[:, :, TD:ED].reshape(P, NB * H)))
    h_all[NSLOT, :HC] = 0.0
    h_all[NSLOT, HC:] = PAD_ASRC

    in_maps = []
    for cc in range(NCORES):
        IDX = lay["idxs"][cc]
        he = np.empty((P, TOT), dtype=f16)
        for jb in range(NB):
            Lg = int(L_uni[jb])
            blk = h_all[IDX[:, LOFF[jb]:LOFF[jb + 1]]]       # [P, Lg, TD]
            he[:, BOFF[jb]:BOFF[jb] + TD * Lg] = (
                blk.transpose(0, 2, 1).reshape(P, TD * Lg))
        in_maps.append({"he": he, "adst": adsts[cc]})
    return in_maps


def _unshard_output(res, lay):
    out_full = np.empty((N, HC), dtype=np.float32)
    row2node = lay["row2node"]
    for cc in range(NCORES):
        o3 = (res[cc]["outp"].astype(np.float32)
              .reshape(P, NB, HC).transpose(1, 0, 2).reshape(NPC, HC))
        rr = row2node[cc * NPC:(cc + 1) * NPC]
        m = rr >= 0
        out_full[rr[m]] = o3[m]
    return out_full


def kernel_with_programs(x, edge_index, W, att_src, att_dst, bias):
    lay = _graph_layout(edge_index)
    Wext = _fold_weights(W, att_src, att_dst, bias)

    nc_a = _build_phase_a()
    maps_a = _prep_phase_a_inputs(x, lay, Wext)
    res_a = run_bass_kernel_spmd(nc_a, maps_a, core_ids=list(range(NCORES)))

    nc_b = _build_phase_b(lay["groups"], lay["TOT"], lay["L_real"])
    maps_b = _prep_phase_b_inputs(res_a.results, lay)
    res_b = run_bass_kernel_spmd(nc_b, maps_b, core_ids=list(range(NCORES)))

    return _unshard_output(res_b.results, lay), (nc_a, nc_b)


def kernel(x, edge_index, W, att_src, att_dst, bias):
    out, _ = kernel_with_programs(x, edge_index, W, att_src, att_dst, bias)
    return out
